# revision 32
# baseline (speedup 1.0000x reference)
"""AnimeStyleAttention distributed Bass kernel for 8 TRN2 NeuronCores.

Full module: y = (softmax(q k^T / 8) v  *  gate(style)) @ Wo + bo
  with q/k/v = x @ W{q,k,v} + b,  gate = sigmoid(gelu(style@Ws1+bs1)@Ws2+bs2)

Sharding: core c -> (batch b = c//2, head-group g = c%2): one batch element,
4 of the 8 heads (a 256-channel slice) per core; host sums core pairs.

Host-side prep (make_in_maps):
  * x pre-transposed to the exact [128, 4, 2048] SBUF layout; 16 contiguous
    token-chunk DMAs over the 3 DMA queues, (k, t=0) chunks first.
  * style gate precomputed (float64, exact erf gelu) and folded into Wv/bv
    (out*gate == attn @ (v*gate)); Wo carries the gamma denominator scale.
  * all weights pre-arranged to [128, k, M] SBUF layouts.

Softmax without max-subtraction (|scores| < 1.8 by construction) in a
consistent K-scaled form ee = exp(s)/K, K = 0.0360906 so the same values come
from either engine:
  * ScalarE: exp with per-partition bias ln(1/K) -- the only ACT function, so
    one table load (hoisted to t~0 by a dummy exp).
  * VectorE: a custom 1-pass DVE op (EXP_QUARTIC_ANT, registered at import):
    ((x+a)^2+b)*((x+c)^2+d), the factored minimax quartic of exp(x)/K on
    [-1.85, 1.85] (rel err ~1.0%, end-to-end ~2.8e-3 vs 2e-2 budget).
    Steady-state k-tiles with kt%3==2 use it, splitting the exp wall
    (~1.1us/tile on ScalarE) across two engines.

Per core: scores^T = k q^T as row-packed concurrent K=64 matmul pairs
[k-tok partitions, q-tok free]; attn.v accumulates in PSUM with a 65th
gamma-column (gamma=1/4 keeps the fp16 denominator under 2^16); denominator
row broadcast across partitions by a K=1 ones-matmul, reciprocal_approx_fast,
zT = drain * recip; y = zT^T Wo + bo -> DRAM f16.

Schedule: one flat stream over all (chunk, k-tile) units with a 2-unit score
lookahead, so attn.v never waits on exp and PE matmuls stay back-to-back
across chunk boundaries.  pr=0 chunks run first; the m=1 (heads 2,3) k/q
projections backfill into early chunks' spare PE slots.  Each chunk's
normalization is split: fp16 drains right after the last accumulation,
broadcast/recip/multiply 2 units into the next chunk, output projections one
token-tile at a time at units 4/6/8/10.  The final chunk pipelines its
normalization in two half-width passes.
"""

import math
from contextlib import ExitStack

import numpy as np

import concourse.bacc as bacc
import concourse.bass as bass
import concourse.tile as tile
from concourse import mybir
from concourse import dve_ops as _dvo
from concourse.dve_spec import (
    C0, C1, C2, C3, Spec, Src0, _has_src1, _spill_c3_to_src1, lower as _dve_lower, sq,
)
from concourse.dve_table_gen import dve_ver_for
from concourse.dve_uop import DveOpSpec

P = 128
N = 2048          # tokens (one batch element per core)
D = 512           # model dim
CH = 256          # this core's qkv channel slice (4 heads x 64)
NKT = N // P      # 16 token tiles
QC = 512          # q-chunk width
NQC = N // QC     # 4
F16 = mybir.dt.float16
F32 = mybir.dt.float32
AF = mybir.ActivationFunctionType

# minimax quartic of exp(x)/K on [-1.85, 1.85], factored into two real
# quadratics: exp(x)/K ~ ((x+QA)^2+QB) * ((x+QC_)^2+QD), K = QK
QA, QB = 2.24228120, 0.50837655
QC_, QD = 0.46834678, 4.73542865
QK = 0.0360906022
EXP_BIAS = -math.log(QK)   # ScalarE: exp(x + EXP_BIAS) = exp(x)/K
GAMMA = 0.25               # denominator column value (fp16 headroom)


def _expq_reference(in0, in1, s0, s1, imm2):
    c3 = np.asarray(in1).reshape(np.asarray(in1).shape[0], -1)[:, 0]
    x = np.asarray(in0, dtype=np.float32)
    c3 = c3.reshape((-1,) + (1,) * (x.ndim - 1)).astype(np.float32)
    return (((x + s0) ** 2 + s1) * ((x + imm2) ** 2 + c3)).astype(np.float32)


def _register_expq() -> "_dvo.DveOp":
    name = "EXP_QUARTIC_ANT"
    for op in _dvo.OPS:
        if op.name == name:
            return op
    body = _spill_c3_to_src1((sq(Src0 + C0) + C1) * (sq(Src0 + C2) + C3))
    spec = Spec(body=body, reference=_expq_reference)
    if name not in _dvo._SUB_OPCODE_FOR_NAME:
        _dvo._SUB_OPCODE_FOR_NAME[name] = max(_dvo._SUB_OPCODE_FOR_NAME.values()) + 1
    assert _dvo._SUB_OPCODE_FOR_NAME[name] < 0x20
    shas = {}
    for ver in ("v3", "v4"):
        try:
            shas[ver] = DveOpSpec(
                name=name,
                opcode=_dvo.get_dve_sub_opcode(name),
                uops=_dve_lower(spec, ver=ver),
                rd1_en=_has_src1(spec),
            ).sha(ver)
        except Exception:
            pass
    op = _dvo.DveOp(name, spec, subdim=False, uops_sha=shas)
    _dvo.OPS.append(op)
    _dvo.CUSTOM_DVE_SPECS[name] = spec
    return op


_EXPQ = _register_expq()


def build_program() -> bass.Bass:
    nc = bacc.Bacc()

    xt_d = nc.declare_dram_parameter("xt", [P, 4 * N], F16, isOutput=False)
    wq_d = nc.declare_dram_parameter("wq", [P, 4 * CH], F16, isOutput=False)
    wk_d = nc.declare_dram_parameter("wk", [P, 4 * CH], F16, isOutput=False)
    wv_d = nc.declare_dram_parameter("wv", [P, 4 * CH], F16, isOutput=False)
    wo_d = nc.declare_dram_parameter("wo", [P, 2 * D], F16, isOutput=False)
    sm_d = nc.declare_dram_parameter("smalls", [P, 4], F32, isOutput=False)
    bv_d = nc.declare_dram_parameter("bv", [CH], F32, isOutput=False)
    bo_d = nc.declare_dram_parameter("bo", [D], F32, isOutput=False)
    out_d = nc.declare_dram_parameter("out", [N, D], F16, isOutput=True)

    with ExitStack() as ctx:
        tc = ctx.enter_context(tile.TileContext(nc))
        const = ctx.enter_context(tc.tile_pool(name="const", bufs=1))

        # ---- input DMAs: token-chunk-first over the 3 DMA-capable queues ----
        xts = const.tile([P, 4, N], F16)
        qdma = [nc.sync, nc.scalar, nc.gpsimd, nc.sync]
        wq = const.tile([P, 2, 4, P], F16)
        wk = const.tile([P, 2, 4, P], F16)
        wv = const.tile([P, 4, CH], F16)
        wo = const.tile([P, 2, D], F16)
        smalls = const.tile([P, 4], F32)
        bvb = const.tile([P, CH], F32)
        bob = const.tile([P, D], F32)

        def xchunk(k, t):
            sl = slice(t * QC, (t + 1) * QC)
            qdma[k].dma_start(xts[:, k, sl], xt_d[:, k * N + t * QC : k * N + (t + 1) * QC])

        # wq/wk are m-major ([P, m, k, 128]) so the m=0 halves (all phase-1
        # needs) are single contiguous half-size DMAs arriving first
        wkr = wk_d.rearrange("p (m k c) -> p m k c", m=2, k=4)
        wqr = wq_d.rearrange("p (m k c) -> p m k c", m=2, k=4)
        nc.gpsimd.dma_start(wk[:, 0], wkr[:, 0])
        xchunk(0, 0)
        xchunk(1, 0)
        xchunk(2, 0)
        xchunk(3, 0)
        nc.scalar.dma_start(smalls, sm_d[:, 0:4])
        nc.gpsimd.dma_start(wq[:, 0], wqr[:, 0])
        nc.sync.dma_start(wv, wv_d.rearrange("p (k m) -> p k m", k=4))
        nc.sync.dma_start(bvb, bv_d.rearrange("(o c) -> o c", o=1).to_broadcast((P, CH)))

        # dummy 1-elem exp: hoists the single exp ACT_TABLE_LOAD to t~0
        dumm = const.tile([1, 1], F16)
        nc.vector.memset(dumm, 0.0)
        dume = const.tile([1, 1], F16)
        nc.scalar.activation(dume, dumm, AF.Exp)
        # warmup source for the PE-clock (HAM) dummy matmuls
        dsrc = const.tile([64, QC], F16)
        nc.vector.memset(dsrc, 0.0)

        xchunk(1, 1)
        xchunk(2, 1)
        nc.gpsimd.dma_start(wk[:, 1], wkr[:, 1])
        nc.gpsimd.dma_start(wq[:, 1], wqr[:, 1])
        for t in range(1, 4):
            xchunk(0, t)
            xchunk(3, t)
        for t in range(2, 4):
            xchunk(1, t)
            xchunk(2, t)
        nc.gpsimd.dma_start(wo, wo_d.rearrange("p (k m) -> p k m", k=2))
        nc.sync.dma_start(bob, bo_d.rearrange("(o c) -> o c", o=1).to_broadcast((P, D)))

        bqT = smalls[:, 0:2]
        bkT = smalls[:, 2:4]

        ones_row = const.tile([1, 64], F16)
        nc.vector.memset(ones_row, 1.0)
        dconst = const.tile([P, 1], F32)
        nc.vector.memset(dconst, QD)
        ebias = const.tile([P, 1], F32)
        nc.vector.memset(ebias, EXP_BIAS)

        qT = const.tile([P, 2, N], F16)
        kT = const.tile([P, 2, N], F16)
        vv = const.tile([P, NKT, 4, 65], F16)  # 64 gated-v cols + gamma col
        nc.vector.memset(vv[:, :, :, 64:65], GAMMA)
        zT = const.tile([P, 2, N], F16)

        with (
            tc.tile_pool(name="scps", bufs=2, space="PSUM") as scps,
            tc.tile_pool(name="ops", bufs=2, space="PSUM") as ops,
            tc.tile_pool(name="mps", bufs=2, space="PSUM") as mps,
            tc.tile_pool(name="esb", bufs=8) as esb,
            tc.tile_pool(name="rsb", bufs=6) as rsb,
            tc.tile_pool(name="ysb", bufs=3) as ysb,
        ):
            def emit_qk(dst, w, bias, m, qc):
                s_ = slice(qc * QC, (qc + 1) * QC)
                ps = mps.tile([P, QC], F32, tag="m")
                for k in range(4):
                    nc.tensor.matmul(
                        ps,
                        lhsT=w[:, m, k, :],
                        rhs=xts[:, k, s_],
                        start=(k == 0),
                        stop=(k == 3),
                    )
                nc.vector.tensor_scalar_add(dst[:, m, s_], ps, bias[:, m : m + 1])

            def emit_vproj(tt):
                psv_full = mps.tile([P, QC], F32, tag="m")
                psv = psv_full[:, 0:CH]
                for k in range(4):
                    nc.tensor.matmul(
                        psv,
                        lhsT=xts[:, k, tt * P : (tt + 1) * P],
                        rhs=wv[:, k, :],
                        start=(k == 0),
                        stop=(k == 3),
                    )
                nc.vector.tensor_add(
                    vv[:, tt, :, 0:64],
                    psv.rearrange("p (h w) -> p h w", w=64),
                    bvb.rearrange("p (h w) -> p h w", w=64),
                )

            def emit_scores_exp(pr, qc, kt, on_dve):
                s = slice(qc * QC, (qc + 1) * QC)
                ks = slice(kt * P, (kt + 1) * P)
                ee = esb.tile([P, 2, QC], F16, tag="e")
                if on_dve:
                    # DVE-exp'd tiles keep their scores out of the scps
                    # ping-pong (two 1-bank tiles) so the ScalarE exp stream
                    # can run ahead instead of lockstepping with attn.v
                    slo = mps.tile([P, QC], F32, tag="m")
                    shi = mps.tile([P, QC], F32, tag="m")
                    nc.tensor.matmul(
                        slo, lhsT=kT[0:64, pr, ks], rhs=qT[0:64, pr, s],
                        start=True, stop=True,
                    )
                    nc.tensor.matmul(
                        shi, lhsT=kT[64:128, pr, ks], rhs=qT[64:128, pr, s],
                        start=True, stop=True,
                    )
                    nc.vector._custom_dve(
                        _EXPQ, out=ee[:, 0, :], in0=slo, in1=dconst,
                        s0=QA, s1=QB, imm2=QC_,
                    )
                    nc.vector._custom_dve(
                        _EXPQ, out=ee[:, 1, :], in0=shi, in1=dconst,
                        s0=QA, s1=QB, imm2=QC_,
                    )
                else:
                    sc = scps.tile([P, 2, QC], F32, tag="sc")
                    nc.tensor.matmul(
                        sc[:, 0, :], lhsT=kT[0:64, pr, ks], rhs=qT[0:64, pr, s],
                        start=True, stop=True,
                    )
                    nc.tensor.matmul(
                        sc[:, 1, :], lhsT=kT[64:128, pr, ks], rhs=qT[64:128, pr, s],
                        start=True, stop=True,
                    )
                    nc.scalar.activation(ee, sc, AF.Exp, bias=ebias[:, 0:1])
                return ee

            def emit_attnv(pr, out_lo, out_hi, kt, ee):
                nc.tensor.matmul(
                    out_lo, lhsT=vv[:, kt, 2 * pr, :], rhs=ee[:, 0, :],
                    start=(kt == 0), stop=(kt == NKT - 1),
                )
                nc.tensor.matmul(
                    out_hi, lhsT=vv[:, kt, 2 * pr + 1, :], rhs=ee[:, 1, :],
                    start=(kt == 0), stop=(kt == NKT - 1),
                )

            def emit_zt_pre(out_lo, out_hi, cols=slice(0, QC)):
                # fp16 drains: denominator rows to partition-0 tiles (for the
                # broadcast matmul rhs), attn output halves stacked into one
                # [128, QC] tile (gate folded into v on the host); frees the
                # accumulation banks for the next chunk-pair
                den_l = rsb.tile([1, QC], F16, tag="r")
                den_h = rsb.tile([1, QC], F16, tag="r")
                od = rsb.tile([P, QC], F16, tag="od")
                # od casts first: the next chunk-pair's attn.v reuses these
                # banks, while the den rows only gate the broadcast 2+ units in
                with nc.allow_low_precision(reason="fp16 out/denom O(1e4)"):
                    nc.vector.tensor_copy(od[0:64, cols], out_lo[0:64, cols])
                    nc.vector.tensor_copy(od[64:128, cols], out_hi[0:64, cols])
                    nc.vector.tensor_copy(den_l[:, cols], out_lo[64:65, cols])
                    nc.vector.tensor_copy(den_h[:, cols], out_hi[64:65, cols])
                return den_l, den_h, od

            def emit_zt_post(pr, qc, den_l, den_h, od, cols=slice(0, QC)):
                s = slice(qc * QC + cols.start, qc * QC + cols.stop)
                w_ = cols.stop - cols.start
                rcb_ps = mps.tile([P, QC], F32, tag="m")
                nc.tensor.matmul(
                    rcb_ps[0:64, 0:w_], lhsT=ones_row, rhs=den_l[:, cols],
                    start=True, stop=True, tile_position=(0, 0),
                )
                nc.tensor.matmul(
                    rcb_ps[64:128, 0:w_], lhsT=ones_row, rhs=den_h[:, cols],
                    start=True, stop=True, tile_position=(0, 64),
                )
                rcb = rsb.tile([P, QC], F32, tag="rc")
                nc.vector.reciprocal_approx_fast(out=rcb[:, 0:w_], in_=rcb_ps[:, 0:w_])
                nc.vector.tensor_mul(zT[:, pr, s], od[:, cols], rcb[:, 0:w_])

            def emit_outproj_tt(tt):
                ps = mps.tile([P, D], F32, tag="m")
                for m in range(2):
                    nc.tensor.matmul(
                        ps,
                        lhsT=zT[:, m, tt * P : (tt + 1) * P],
                        rhs=wo[:, m, :],
                        start=(m == 0),
                        stop=(m == 1),
                    )
                y = ysb.tile([P, D], F16, tag="ys")
                with nc.allow_low_precision(reason="fp16 output, tol 2e-2"):
                    nc.vector.tensor_add(y, ps, bob)
                nc.sync.dma_start(out_d[tt * P : (tt + 1) * P, :], y)

            # ---- flat software-pipelined stream over all (chunk, kt) units:
            #      pr=0 chunks first so the m=1 projections can backfill ----
            cps = [(0, 0), (1, 0), (2, 0), (3, 0), (0, 1), (1, 1), (2, 1), (3, 1)]
            units = [(i, qc, pr, kt) for i, (qc, pr) in enumerate(cps) for kt in range(NKT)]
            # PE backfill work (~0.9us each), scheduled at specific units:
            #   phase-1 (i=0): kT m0 c1..3 + qT m0 c1 early, v-proj one per unit
            #   i=1..3: remaining q/k m=1 projections, >=1 chunk before use
            fills = {
                (0, 0): lambda: emit_qk(kT, wk, bkT, 0, 1),
                (0, 1): lambda: emit_qk(kT, wk, bkT, 0, 2),
                (0, 2): lambda: emit_qk(kT, wk, bkT, 0, 3),
                (0, 4): lambda: emit_qk(qT, wq, bqT, 0, 1),
                (1, 5): lambda: emit_qk(qT, wq, bqT, 0, 2),
                (1, 7): lambda: emit_qk(kT, wk, bkT, 1, 0),
                (1, 9): lambda: emit_qk(kT, wk, bkT, 1, 1),
                (1, 11): lambda: emit_qk(kT, wk, bkT, 1, 2),
                (1, 13): lambda: emit_qk(kT, wk, bkT, 1, 3),
                (2, 5): lambda: emit_qk(qT, wq, bqT, 0, 3),
                (2, 7): lambda: emit_qk(qT, wq, bqT, 1, 0),
                (2, 9): lambda: emit_qk(qT, wq, bqT, 1, 1),
                (3, 5): lambda: emit_qk(qT, wq, bqT, 1, 2),
                (3, 7): lambda: emit_qk(qT, wq, bqT, 1, 3),
            }

            def sc_unit(u):
                i, qc, pr, kt = units[u]
                on_dve = (i >= 1) and (kt % 4 == 2)
                return emit_scores_exp(pr, qc, kt, on_dve)

            # PE clock (HAM) warmup: dummy matmuls in the input-DMA shadow
            # keep the PE activity window busy so phase-1 runs at 2.4 GHz
            def emit_warmup(n):
                for _ in range(n):
                    dps = mps.tile([P, QC], F32, tag="m")
                    nc.tensor.matmul(
                        dps[0:64, :], lhsT=dsrc[:, 0:64], rhs=dsrc,
                        start=True, stop=True,
                    )

            emit_warmup(6)
            emit_qk(kT, wk, bkT, 0, 0)
            emit_qk(qT, wq, bqT, 0, 0)
            ee = {0: sc_unit(0), 1: sc_unit(1)}
            out_lo = out_hi = None
            pending = None        # (pr, qc, od_lo, od_hi) awaiting zt_post
            outproj_q = []        # token tiles awaiting output projection
            for u, (i, qc, pr, kt) in enumerate(units):
                if kt == 0:
                    out_lo = ops.tile([65, QC], F32, tag="o")
                    out_hi = ops.tile([65, QC], F32, tag="o")
                if (i, kt) in fills:
                    fills[(i, kt)]()
                if i == 0:
                    emit_vproj(kt)
                emit_attnv(pr, out_lo, out_hi, kt, ee.pop(u))
                if u + 2 < len(units):
                    ee[u + 2] = sc_unit(u + 2)
                if kt == 2 and pending is not None:
                    emit_zt_post(*pending)
                    pending = None
                    if cps[i - 1][1] == 1:  # chunk qc of (qc, pr=1) complete
                        outproj_q = list(range(cps[i - 1][0] * 4, cps[i - 1][0] * 4 + 4))
                if kt in (6, 10, 14, 15) and outproj_q:
                    emit_outproj_tt(outproj_q.pop(0))
                if kt == NKT - 1 and u + 1 < len(units):
                    den_l, den_h, od = emit_zt_pre(out_lo, out_hi)
                    pending = (pr, qc, den_l, den_h, od)

            # ---- tail: final chunk normalization + output projection in two
            #      half-width passes to pipeline DVE against PE ----
            for h in range(2):
                cols = slice(h * (QC // 2), (h + 1) * (QC // 2))
                den_l, den_h, od = emit_zt_pre(out_lo, out_hi, cols)
                emit_zt_post(1, NQC - 1, den_l, den_h, od, cols)
                for tt in ((NQC - 1) * 4 + 2 * h, (NQC - 1) * 4 + 2 * h + 1):
                    emit_outproj_tt(tt)

    nc.finalize()
    return nc


_NC_CACHE = None


def _get_program() -> bass.Bass:
    global _NC_CACHE
    if _NC_CACHE is None:
        _NC_CACHE = build_program()
    return _NC_CACHE


def _host_gate(inputs: dict) -> np.ndarray:
    """sigmoid(gelu(style@Ws1+bs1, exact erf)@Ws2+bs2) in float64 -> [4, D]."""
    erf = np.frompyfunc(math.erf, 1, 1)
    h = inputs["style"].astype(np.float64) @ inputs["Ws1"].astype(np.float64)
    h = h + inputs["bs1"].astype(np.float64)
    g = h * 0.5 * (1.0 + erf(h / math.sqrt(2.0)).astype(np.float64))
    z = g @ inputs["Ws2"].astype(np.float64) + inputs["bs2"].astype(np.float64)
    return 1.0 / (1.0 + np.exp(-z))


def _to_sbuf_layout(w: np.ndarray, k: int) -> np.ndarray:
    """[k*128, M] -> [128, k*M] matching a [128, k, M] SBUF tile."""
    km = w.shape[1]
    return np.ascontiguousarray(
        w.reshape(k, P, km).transpose(1, 0, 2).reshape(P, k * km)
    )


def _to_sbuf_layout_mmaj(w: np.ndarray) -> np.ndarray:
    """[512, 256] -> [128, m(2)*k(4)*128] matching a [128, 2, 4, 128] tile."""
    return np.ascontiguousarray(
        w.reshape(4, P, 2, P).transpose(1, 2, 0, 3).reshape(P, 8 * P)
    )


def make_in_maps(inputs: dict) -> list[dict]:
    f16 = np.float16
    f32 = np.float32
    scale = 1.0 / 8.0  # 1/sqrt(head_dim), folded into Wq/bq
    x = inputs["x"]
    gate = _host_gate(inputs)
    in_maps = []
    for c in range(8):
        b, g = divmod(c, 2)
        ch = slice(CH * g, CH * (g + 1))
        gt = gate[b, ch]
        smalls = np.stack(
            [
                inputs["bq"][ch].reshape(2, P) * scale,
                inputs["bk"][ch].reshape(2, P),
            ]
        ).reshape(4, P)
        in_maps.append(
            {
                "xt": _to_sbuf_layout(x[b].T.astype(f16), 4),
                "wq": _to_sbuf_layout_mmaj((inputs["Wq"][:, ch] * scale).astype(f16)),
                "wk": _to_sbuf_layout_mmaj(inputs["Wk"][:, ch].astype(f16)),
                "wv": _to_sbuf_layout(
                    (inputs["Wv"][:, ch].astype(np.float64) * gt[None, :]).astype(f16), 4
                ),
                # gamma-scaled: zT carries 1/gamma from the denominator column
                "wo": _to_sbuf_layout(
                    (inputs["Wo"][ch, :].astype(np.float64) * GAMMA).astype(f16), 2
                ),
                "smalls": np.ascontiguousarray(smalls.T).astype(f32),
                "bv": np.ascontiguousarray(
                    inputs["bv"][ch].astype(np.float64) * gt
                ).astype(f32),
                "bo": (
                    np.ascontiguousarray(inputs["bo"]).astype(f32)
                    if g == 0
                    else np.zeros_like(inputs["bo"], dtype=f32)
                ),
            }
        )
    return in_maps


def kernel(**inputs) -> np.ndarray:
    from concourse.bass_utils import run_bass_kernel_spmd

    in_maps = make_in_maps(inputs)
    res = run_bass_kernel_spmd(_get_program(), in_maps, list(range(8))).results
    y = np.stack(
        [
            res[2 * b]["out"].astype(np.float32)
            + res[2 * b + 1]["out"].astype(np.float32)
            for b in range(4)
        ]
    )
    return y


# revision 33
# speedup vs baseline: 1.1927x; 1.1927x over previous
"""AnimeStyleAttention distributed Bass kernel for 8 TRN2 NeuronCores.

Full module: y = (softmax(q k^T / 8) v  *  gate(style)) @ Wo + bo
  with q/k/v = x @ W{q,k,v} + b,  gate = sigmoid(gelu(style@Ws1+bs1)@Ws2+bs2)

Sharding: core c -> (batch b = c//2, head-group g = c%2): one batch element,
4 of the 8 heads (a 256-channel slice) per core; host sums core pairs.

Host-side prep (make_in_maps):
  * x pre-transposed to the exact [128, 4, 2048] SBUF layout; 16 contiguous
    token-chunk DMAs over the 3 DMA queues, (k, t=0) chunks first.
  * style gate precomputed (float64, exact erf gelu) and folded into Wv/bv
    (out*gate == attn @ (v*gate)); Wo carries the gamma denominator scale.
  * all weights pre-arranged to [128, k, M] SBUF layouts.

Softmax without max-subtraction (|scores| < 1.8 by construction) in a
consistent K-scaled form ee = exp(s)/K, K = 0.0360906 so the same values come
from either engine:
  * ScalarE: exp with per-partition bias ln(1/K) -- the only ACT function, so
    one table load (hoisted to t~0 by a dummy exp).
  * VectorE: a custom 1-pass DVE op (EXP_QUARTIC_ANT, registered at import):
    ((x+a)^2+b)*((x+c)^2+d), the factored minimax quartic of exp(x)/K on
    [-1.85, 1.85] (rel err ~1.0%, end-to-end ~2.8e-3 vs 2e-2 budget).
    Steady-state k-tiles with kt%3==2 use it, splitting the exp wall
    (~1.1us/tile on ScalarE) across two engines.

Per core: scores^T = k q^T as row-packed concurrent K=64 matmul pairs
[k-tok partitions, q-tok free]; attn.v accumulates in PSUM with a 65th
gamma-column (gamma=1/4 keeps the fp16 denominator under 2^16); denominator
row broadcast across partitions by a K=1 ones-matmul, reciprocal_approx_fast,
zT = drain * recip; y = zT^T Wo + bo -> DRAM f16.

Schedule: one flat stream over all (chunk, k-tile) units with a 2-unit score
lookahead, so attn.v never waits on exp and PE matmuls stay back-to-back
across chunk boundaries.  pr=0 chunks run first; the m=1 (heads 2,3) k/q
projections backfill into early chunks' spare PE slots.  Each chunk's
normalization is split: fp16 drains right after the last accumulation,
broadcast/recip/multiply 2 units into the next chunk, output projections one
token-tile at a time at units 4/6/8/10.  The final chunk pipelines its
normalization in two half-width passes.
"""

import math
from contextlib import ExitStack

import numpy as np

import concourse.bacc as bacc
import concourse.bass as bass
import concourse.tile as tile
from concourse import mybir
from concourse import dve_ops as _dvo
from concourse.dve_spec import (
    C0, C1, C2, C3, Spec, Src0, _has_src1, _spill_c3_to_src1, lower as _dve_lower, sq,
)
from concourse.dve_table_gen import dve_ver_for
from concourse.dve_uop import DveOpSpec

P = 128
N = 2048          # tokens (one batch element per core)
D = 512           # model dim
CH = 256          # this core's qkv channel slice (4 heads x 64)
NKT = N // P      # 16 token tiles
QC = 512          # q-chunk width
NQC = N // QC     # 4
F16 = mybir.dt.float16
F32 = mybir.dt.float32
AF = mybir.ActivationFunctionType

# minimax quartic of exp(x)/K on [-1.85, 1.85], factored into two real
# quadratics: exp(x)/K ~ ((x+QA)^2+QB) * ((x+QC_)^2+QD), K = QK
QA, QB = 2.24228120, 0.50837655
QC_, QD = 0.46834678, 4.73542865
QK = 0.0360906022
EXP_BIAS = -math.log(QK)   # ScalarE: exp(x + EXP_BIAS) = exp(x)/K
GAMMA = 0.25               # denominator column value (fp16 headroom)


def _expq_reference(in0, in1, s0, s1, imm2):
    c3 = np.asarray(in1).reshape(np.asarray(in1).shape[0], -1)[:, 0]
    x = np.asarray(in0, dtype=np.float32)
    c3 = c3.reshape((-1,) + (1,) * (x.ndim - 1)).astype(np.float32)
    return (((x + s0) ** 2 + s1) * ((x + imm2) ** 2 + c3)).astype(np.float32)


def _register_expq() -> "_dvo.DveOp":
    name = "EXP_QUARTIC_ANT"
    for op in _dvo.OPS:
        if op.name == name:
            return op
    body = _spill_c3_to_src1((sq(Src0 + C0) + C1) * (sq(Src0 + C2) + C3))
    spec = Spec(body=body, reference=_expq_reference)
    if name not in _dvo._SUB_OPCODE_FOR_NAME:
        _dvo._SUB_OPCODE_FOR_NAME[name] = max(_dvo._SUB_OPCODE_FOR_NAME.values()) + 1
    assert _dvo._SUB_OPCODE_FOR_NAME[name] < 0x20
    shas = {}
    for ver in ("v3", "v4"):
        try:
            shas[ver] = DveOpSpec(
                name=name,
                opcode=_dvo.get_dve_sub_opcode(name),
                uops=_dve_lower(spec, ver=ver),
                rd1_en=_has_src1(spec),
            ).sha(ver)
        except Exception:
            pass
    op = _dvo.DveOp(name, spec, subdim=False, uops_sha=shas)
    _dvo.OPS.append(op)
    _dvo.CUSTOM_DVE_SPECS[name] = spec
    return op


_EXPQ = _register_expq()


def build_program() -> bass.Bass:
    nc = bacc.Bacc()

    xt_d = nc.declare_dram_parameter("xt", [P, 4 * N], F16, isOutput=False)
    wq_d = nc.declare_dram_parameter("wq", [P, 4 * CH], F16, isOutput=False)
    wk_d = nc.declare_dram_parameter("wk", [P, 4 * CH], F16, isOutput=False)
    wv_d = nc.declare_dram_parameter("wv", [P, 4 * CH], F16, isOutput=False)
    wo_d = nc.declare_dram_parameter("wo", [P, 2 * D], F16, isOutput=False)
    sm_d = nc.declare_dram_parameter("smalls", [P, 4], F32, isOutput=False)
    bv_d = nc.declare_dram_parameter("bv", [CH], F32, isOutput=False)
    bo_d = nc.declare_dram_parameter("bo", [D], F32, isOutput=False)
    out_d = nc.declare_dram_parameter("out", [N, D], F16, isOutput=True)

    with ExitStack() as ctx:
        tc = ctx.enter_context(tile.TileContext(nc))
        const = ctx.enter_context(tc.tile_pool(name="const", bufs=1))

        # ---- input DMAs: token-chunk-first over the 3 DMA-capable queues ----
        xts = const.tile([P, 4, N], F16)
        qdma = [nc.sync, nc.scalar, nc.gpsimd, nc.sync]
        wq = const.tile([P, 2, 4, P], F16)
        wk = const.tile([P, 2, 4, P], F16)
        wv = const.tile([P, 4, CH], F16)
        wo = const.tile([P, 2, D], F16)
        smalls = const.tile([P, 4], F32)
        bvb = const.tile([P, CH], F32)
        bob = const.tile([P, D], F32)

        def xchunk(k, t):
            sl = slice(t * QC, (t + 1) * QC)
            qdma[k].dma_start(xts[:, k, sl], xt_d[:, k * N + t * QC : k * N + (t + 1) * QC])

        # wq/wk are m-major ([P, m, k, 128]) so the m=0 halves (all phase-1
        # needs) are single contiguous half-size DMAs arriving first
        wkr = wk_d.rearrange("p (m k c) -> p m k c", m=2, k=4)
        wqr = wq_d.rearrange("p (m k c) -> p m k c", m=2, k=4)
        nc.gpsimd.dma_start(wk[:, 0], wkr[:, 0])
        xchunk(0, 0)
        xchunk(1, 0)
        xchunk(2, 0)
        xchunk(3, 0)
        nc.scalar.dma_start(smalls, sm_d[:, 0:4])
        nc.gpsimd.dma_start(wq[:, 0], wqr[:, 0])
        nc.sync.dma_start(wv, wv_d.rearrange("p (k m) -> p k m", k=4))
        nc.sync.dma_start(bvb, bv_d.rearrange("(o c) -> o c", o=1).to_broadcast((P, CH)))

        # dummy 1-elem exp: hoists the single exp ACT_TABLE_LOAD to t~0
        dumm = const.tile([1, 1], F16)
        nc.vector.memset(dumm, 0.0)
        dume = const.tile([1, 1], F16)
        nc.scalar.activation(dume, dumm, AF.Exp)
        # warmup source for the PE-clock (HAM) dummy matmuls
        dsrc = const.tile([64, QC], F16)
        nc.vector.memset(dsrc, 0.0)

        xchunk(1, 1)
        xchunk(2, 1)
        nc.gpsimd.dma_start(wk[:, 1], wkr[:, 1])
        nc.gpsimd.dma_start(wq[:, 1], wqr[:, 1])
        for t in range(1, 4):
            xchunk(0, t)
            xchunk(3, t)
        for t in range(2, 4):
            xchunk(1, t)
            xchunk(2, t)
        nc.gpsimd.dma_start(wo, wo_d.rearrange("p (k m) -> p k m", k=2))
        nc.sync.dma_start(bob, bo_d.rearrange("(o c) -> o c", o=1).to_broadcast((P, D)))

        bqT = smalls[:, 0:2]
        bkT = smalls[:, 2:4]

        ones_row = const.tile([1, 64], F16)
        nc.vector.memset(ones_row, 1.0)
        dconst = const.tile([P, 1], F32)
        nc.vector.memset(dconst, QD)
        ebias = const.tile([P, 1], F32)
        nc.vector.memset(ebias, EXP_BIAS)

        qT = const.tile([P, 2, N], F16)
        kT = const.tile([P, 2, N], F16)
        vv = const.tile([P, NKT, 4, 65], F16)  # 64 gated-v cols + gamma col
        nc.vector.memset(vv[:, :, :, 64:65], GAMMA)
        zT = const.tile([P, 2, N], F16)

        with (
            tc.tile_pool(name="scps", bufs=2, space="PSUM") as scps,
            tc.tile_pool(name="ops", bufs=2, space="PSUM") as ops,
            tc.tile_pool(name="mps", bufs=2, space="PSUM") as mps,
            tc.tile_pool(name="esb", bufs=8) as esb,
            tc.tile_pool(name="rsb", bufs=6) as rsb,
            tc.tile_pool(name="ysb", bufs=3) as ysb,
        ):
            def emit_qk(dst, w, bias, m, qc):
                s_ = slice(qc * QC, (qc + 1) * QC)
                ps = mps.tile([P, QC], F32, tag="m")
                for k in range(4):
                    nc.tensor.matmul(
                        ps,
                        lhsT=w[:, m, k, :],
                        rhs=xts[:, k, s_],
                        start=(k == 0),
                        stop=(k == 3),
                    )
                nc.vector.tensor_scalar_add(dst[:, m, s_], ps, bias[:, m : m + 1])

            def emit_vproj(tt):
                psv_full = mps.tile([P, QC], F32, tag="m")
                psv = psv_full[:, 0:CH]
                for k in range(4):
                    nc.tensor.matmul(
                        psv,
                        lhsT=xts[:, k, tt * P : (tt + 1) * P],
                        rhs=wv[:, k, :],
                        start=(k == 0),
                        stop=(k == 3),
                    )
                nc.vector.tensor_add(
                    vv[:, tt, :, 0:64],
                    psv.rearrange("p (h w) -> p h w", w=64),
                    bvb.rearrange("p (h w) -> p h w", w=64),
                )

            def emit_scores_exp(pr, qc, kt, on_dve):
                s = slice(qc * QC, (qc + 1) * QC)
                ks = slice(kt * P, (kt + 1) * P)
                ee = esb.tile([P, 2, QC], F16, tag="e")
                if on_dve:
                    # DVE-exp'd tiles keep their scores out of the scps
                    # ping-pong (two 1-bank tiles) so the ScalarE exp stream
                    # can run ahead instead of lockstepping with attn.v
                    slo = mps.tile([P, QC], F32, tag="m")
                    shi = mps.tile([P, QC], F32, tag="m")
                    nc.tensor.matmul(
                        slo, lhsT=kT[0:64, pr, ks], rhs=qT[0:64, pr, s],
                        start=True, stop=True,
                    )
                    nc.tensor.matmul(
                        shi, lhsT=kT[64:128, pr, ks], rhs=qT[64:128, pr, s],
                        start=True, stop=True,
                    )
                    nc.vector._custom_dve(
                        _EXPQ, out=ee[:, 0, :], in0=slo, in1=dconst,
                        s0=QA, s1=QB, imm2=QC_,
                    )
                    nc.vector._custom_dve(
                        _EXPQ, out=ee[:, 1, :], in0=shi, in1=dconst,
                        s0=QA, s1=QB, imm2=QC_,
                    )
                else:
                    sc = scps.tile([P, 2, QC], F32, tag="sc")
                    nc.tensor.matmul(
                        sc[:, 0, :], lhsT=kT[0:64, pr, ks], rhs=qT[0:64, pr, s],
                        start=True, stop=True,
                    )
                    nc.tensor.matmul(
                        sc[:, 1, :], lhsT=kT[64:128, pr, ks], rhs=qT[64:128, pr, s],
                        start=True, stop=True,
                    )
                    nc.scalar.activation(ee, sc, AF.Exp, bias=ebias[:, 0:1])
                return ee

            def emit_attnv(pr, out_lo, out_hi, kt, ee):
                nc.tensor.matmul(
                    out_lo, lhsT=vv[:, kt, 2 * pr, :], rhs=ee[:, 0, :],
                    start=(kt == 0), stop=(kt == NKT - 1),
                )
                nc.tensor.matmul(
                    out_hi, lhsT=vv[:, kt, 2 * pr + 1, :], rhs=ee[:, 1, :],
                    start=(kt == 0), stop=(kt == NKT - 1),
                )

            def emit_zt_pre(out_lo, out_hi, cols=slice(0, QC)):
                # fp16 drains: denominator rows to partition-0 tiles (for the
                # broadcast matmul rhs), attn output halves stacked into one
                # [128, QC] tile (gate folded into v on the host); frees the
                # accumulation banks for the next chunk-pair
                den_l = rsb.tile([1, QC], F16, tag="r")
                den_h = rsb.tile([1, QC], F16, tag="r")
                od = rsb.tile([P, QC], F16, tag="od")
                # od casts first: the next chunk-pair's attn.v reuses these
                # banks, while the den rows only gate the broadcast 2+ units in
                with nc.allow_low_precision(reason="fp16 out/denom O(1e4)"):
                    nc.vector.tensor_copy(od[0:64, cols], out_lo[0:64, cols])
                    nc.vector.tensor_copy(od[64:128, cols], out_hi[0:64, cols])
                    nc.vector.tensor_copy(den_l[:, cols], out_lo[64:65, cols])
                    nc.vector.tensor_copy(den_h[:, cols], out_hi[64:65, cols])
                return den_l, den_h, od

            def emit_zt_post(pr, qc, den_l, den_h, od, cols=slice(0, QC)):
                s = slice(qc * QC + cols.start, qc * QC + cols.stop)
                w_ = cols.stop - cols.start
                rcb_ps = mps.tile([P, QC], F32, tag="m")
                nc.tensor.matmul(
                    rcb_ps[0:64, 0:w_], lhsT=ones_row, rhs=den_l[:, cols],
                    start=True, stop=True, tile_position=(0, 0),
                )
                nc.tensor.matmul(
                    rcb_ps[64:128, 0:w_], lhsT=ones_row, rhs=den_h[:, cols],
                    start=True, stop=True, tile_position=(0, 64),
                )
                rcb = rsb.tile([P, QC], F32, tag="rc")
                nc.vector.reciprocal_approx_fast(out=rcb[:, 0:w_], in_=rcb_ps[:, 0:w_])
                nc.vector.tensor_mul(zT[:, pr, s], od[:, cols], rcb[:, 0:w_])

            def emit_outproj_tt(tt):
                ps = mps.tile([P, D], F32, tag="m")
                for m in range(2):
                    nc.tensor.matmul(
                        ps,
                        lhsT=zT[:, m, tt * P : (tt + 1) * P],
                        rhs=wo[:, m, :],
                        start=(m == 0),
                        stop=(m == 1),
                    )
                y = ysb.tile([P, D], F16, tag="ys")
                with nc.allow_low_precision(reason="fp16 output, tol 2e-2"):
                    nc.vector.tensor_add(y, ps, bob)
                nc.sync.dma_start(out_d[tt * P : (tt + 1) * P, :], y)

            # ---- flat software-pipelined stream over all (chunk, kt) units:
            #      pr=0 chunks first so the m=1 projections can backfill ----
            cps = [(0, 0), (1, 0), (2, 0), (3, 0), (0, 1), (1, 1), (2, 1), (3, 1)]
            units = [(i, qc, pr, kt) for i, (qc, pr) in enumerate(cps) for kt in range(NKT)]
            # PE backfill work (~0.9us each), scheduled at specific units:
            #   phase-1 (i=0): kT m0 c1..3 + qT m0 c1 early, v-proj one per unit
            #   i=1..3: remaining q/k m=1 projections, >=1 chunk before use
            fills = {
                (0, 0): lambda: emit_qk(kT, wk, bkT, 0, 1),
                (0, 1): lambda: emit_qk(kT, wk, bkT, 0, 2),
                (0, 2): lambda: emit_qk(kT, wk, bkT, 0, 3),
                (0, 4): lambda: emit_qk(qT, wq, bqT, 0, 1),
                (1, 5): lambda: emit_qk(qT, wq, bqT, 0, 2),
                (1, 7): lambda: emit_qk(kT, wk, bkT, 1, 0),
                (1, 9): lambda: emit_qk(kT, wk, bkT, 1, 1),
                (1, 11): lambda: emit_qk(kT, wk, bkT, 1, 2),
                (1, 13): lambda: emit_qk(kT, wk, bkT, 1, 3),
                (2, 5): lambda: emit_qk(qT, wq, bqT, 0, 3),
                (2, 7): lambda: emit_qk(qT, wq, bqT, 1, 0),
                (2, 9): lambda: emit_qk(qT, wq, bqT, 1, 1),
                (3, 5): lambda: emit_qk(qT, wq, bqT, 1, 2),
                (3, 7): lambda: emit_qk(qT, wq, bqT, 1, 3),
            }

            def sc_unit(u):
                i, qc, pr, kt = units[u]
                on_dve = (i >= 1) and (kt % 4 == 2)
                return emit_scores_exp(pr, qc, kt, on_dve)

            # PE clock (HAM) warmup: dummy matmuls in the input-DMA shadow
            # keep the PE activity window busy so phase-1 runs at 2.4 GHz
            def emit_warmup(n):
                for _ in range(n):
                    dps = mps.tile([P, QC], F32, tag="m")
                    nc.tensor.matmul(
                        dps[0:64, :], lhsT=dsrc[:, 0:64], rhs=dsrc,
                        start=True, stop=True,
                    )

            emit_warmup(6)
            emit_qk(kT, wk, bkT, 0, 0)
            emit_qk(qT, wq, bqT, 0, 0)
            ee = {0: sc_unit(0), 1: sc_unit(1)}
            out_lo = out_hi = None
            pending = None        # (pr, qc, od_lo, od_hi) awaiting zt_post
            outproj_q = []        # token tiles awaiting output projection
            for u, (i, qc, pr, kt) in enumerate(units):
                if kt == 0:
                    out_lo = ops.tile([65, QC], F32, tag="o")
                    out_hi = ops.tile([65, QC], F32, tag="o")
                if (i, kt) in fills:
                    fills[(i, kt)]()
                if i == 0:
                    emit_vproj(kt)
                emit_attnv(pr, out_lo, out_hi, kt, ee.pop(u))
                if u + 2 < len(units):
                    ee[u + 2] = sc_unit(u + 2)
                if kt == 2 and pending is not None:
                    emit_zt_post(*pending)
                    pending = None
                    if cps[i - 1][1] == 1:  # chunk qc of (qc, pr=1) complete
                        outproj_q = list(range(cps[i - 1][0] * 4, cps[i - 1][0] * 4 + 4))
                if kt in (5, 7, 9, 11) and outproj_q:
                    emit_outproj_tt(outproj_q.pop(0))
                if kt == NKT - 1 and u + 1 < len(units):
                    den_l, den_h, od = emit_zt_pre(out_lo, out_hi)
                    pending = (pr, qc, den_l, den_h, od)

            # ---- tail: final chunk normalization + output projection in two
            #      half-width passes to pipeline DVE against PE ----
            for h in range(2):
                cols = slice(h * (QC // 2), (h + 1) * (QC // 2))
                den_l, den_h, od = emit_zt_pre(out_lo, out_hi, cols)
                emit_zt_post(1, NQC - 1, den_l, den_h, od, cols)
                for tt in ((NQC - 1) * 4 + 2 * h, (NQC - 1) * 4 + 2 * h + 1):
                    emit_outproj_tt(tt)

    nc.finalize()
    return nc


_NC_CACHE = None


def _get_program() -> bass.Bass:
    global _NC_CACHE
    if _NC_CACHE is None:
        _NC_CACHE = build_program()
    return _NC_CACHE


def _host_gate(inputs: dict) -> np.ndarray:
    """sigmoid(gelu(style@Ws1+bs1, exact erf)@Ws2+bs2) in float64 -> [4, D]."""
    erf = np.frompyfunc(math.erf, 1, 1)
    h = inputs["style"].astype(np.float64) @ inputs["Ws1"].astype(np.float64)
    h = h + inputs["bs1"].astype(np.float64)
    g = h * 0.5 * (1.0 + erf(h / math.sqrt(2.0)).astype(np.float64))
    z = g @ inputs["Ws2"].astype(np.float64) + inputs["bs2"].astype(np.float64)
    return 1.0 / (1.0 + np.exp(-z))


def _to_sbuf_layout(w: np.ndarray, k: int) -> np.ndarray:
    """[k*128, M] -> [128, k*M] matching a [128, k, M] SBUF tile."""
    km = w.shape[1]
    return np.ascontiguousarray(
        w.reshape(k, P, km).transpose(1, 0, 2).reshape(P, k * km)
    )


def _to_sbuf_layout_mmaj(w: np.ndarray) -> np.ndarray:
    """[512, 256] -> [128, m(2)*k(4)*128] matching a [128, 2, 4, 128] tile."""
    return np.ascontiguousarray(
        w.reshape(4, P, 2, P).transpose(1, 2, 0, 3).reshape(P, 8 * P)
    )


def make_in_maps(inputs: dict) -> list[dict]:
    f16 = np.float16
    f32 = np.float32
    scale = 1.0 / 8.0  # 1/sqrt(head_dim), folded into Wq/bq
    x = inputs["x"]
    gate = _host_gate(inputs)
    in_maps = []
    for c in range(8):
        b, g = divmod(c, 2)
        ch = slice(CH * g, CH * (g + 1))
        gt = gate[b, ch]
        smalls = np.stack(
            [
                inputs["bq"][ch].reshape(2, P) * scale,
                inputs["bk"][ch].reshape(2, P),
            ]
        ).reshape(4, P)
        in_maps.append(
            {
                "xt": _to_sbuf_layout(x[b].T.astype(f16), 4),
                "wq": _to_sbuf_layout_mmaj((inputs["Wq"][:, ch] * scale).astype(f16)),
                "wk": _to_sbuf_layout_mmaj(inputs["Wk"][:, ch].astype(f16)),
                "wv": _to_sbuf_layout(
                    (inputs["Wv"][:, ch].astype(np.float64) * gt[None, :]).astype(f16), 4
                ),
                # gamma-scaled: zT carries 1/gamma from the denominator column
                "wo": _to_sbuf_layout(
                    (inputs["Wo"][ch, :].astype(np.float64) * GAMMA).astype(f16), 2
                ),
                "smalls": np.ascontiguousarray(smalls.T).astype(f32),
                "bv": np.ascontiguousarray(
                    inputs["bv"][ch].astype(np.float64) * gt
                ).astype(f32),
                "bo": (
                    np.ascontiguousarray(inputs["bo"]).astype(f32)
                    if g == 0
                    else np.zeros_like(inputs["bo"], dtype=f32)
                ),
            }
        )
    return in_maps


def kernel(**inputs) -> np.ndarray:
    from concourse.bass_utils import run_bass_kernel_spmd

    in_maps = make_in_maps(inputs)
    res = run_bass_kernel_spmd(_get_program(), in_maps, list(range(8))).results
    y = np.stack(
        [
            res[2 * b]["out"].astype(np.float32)
            + res[2 * b + 1]["out"].astype(np.float32)
            for b in range(4)
        ]
    )
    return y
